# revision 38
# baseline (speedup 1.0000x reference)
"""Trainium2 Bass kernel for nn_Conv_6511170421767.

3x3 conv, stride 1, pad 1 on x:(32,128,56,56) with weight:(256,128,3,3),
bias:(256,) -> out:(32,256,56,56), fp32 in/out.

Strategy (data-parallel, 4 images per core on 8 cores):
- Cin=128 is exactly the PE contraction/partition dim. The conv becomes,
  per (output-row-block, Cout-chunk), an accumulation of 9 matmuls (one per
  kernel tap): out[co, pix] += W[dr,dc][ci,co].T @ xpad[ci, shifted pix].
- x is zero-padded once into SBUF as [128, 58, 58] per image; a matmul rhs
  slice [128, (8 rows x 58 stride), 56] walks the padded plane, so no edge
  fix-ups are needed. Borders are zeroed by gpsimd memsets (fp16); the
  interior is overwritten per image by DVE casts (f32 stage -> fp16 pad).
- Matmul operands are fp16 (1 PE cycle/row vs 4 for fp32; operand ranges
  sit safely inside fp16's dynamic range; measured rel err 2.9e-4 vs the
  fp32 reference). Weights are converted to fp16 on the host (a 1.1MB
  constant) and DMA'd directly - no on-device weight casts.
- PSUM tile [128, 448] = one bank; 9 taps accumulate in-bank, then the
  scalar engine adds bias (Identity activation w/ per-partition bias AP)
  while copying PSUM->SBUF, and the result DMAs out on the sync queue.

Schedule notes (from iterative trace analysis; baseline was 117.7us,
this version ~112.9us):
- The MM stream runs at the fp16 streaming wall (~191ns per N=448
  matmul) once warm; all the headroom is at the edges:
  - The two HWDGE rings drain with strict priority (sync starves
    scalar), and a DMA trigger occupies its sequencer ~0.6-0.8us. So the
    matmul-critical x stream gets the sync ring, with x00 first, then
    the weight tap groups (0-2/3-5/6-8, each landing just ahead of its
    consumers), then the remaining x tiles. Outputs (batched per
    row-tile via a rearranged DRAM AP) + activations + bias ride the
    scalar ring. DVE runs only the 28 x casts, in consumption order.
  - The PE HAM clock-gate needs ~3.4us of sustained busy to lift the
    engine from 1.2 to 2.4GHz, and a ragged early stream postpones the
    flip (~4.8us lost in the baseline). 7 dummy matmuls on a zeroed fp16
    tile start as soon as the PE body starts (~7.4us) and bridge to the
    first real MM (~10.8us), so the flip lands right as the real stream
    begins. (PE idle >~1.5us between dummies and stream resets the HAM
    window - the bridge must be gap-free - but each surplus dummy also
    delays the stream ~0.37us, so 7 is the measured sweet spot.)
  - Tail: the final Cout chunk is computed as two 4-row PSUM groups so
    its activation+store pipeline is half as deep, with the two output
    DMAs split across both HWDGE rings.
  - Head DMAs must stay few and decoupled: splitting x00 into halves or
    moving it to SWDGE measured SLOWER (semaphore-threshold coupling
    delays the first cast; SWDGE adds ~1us emission+receipt).

The external neuronxcc walrus in this container enforces small per-
instruction sync-wait limits (Matmult/S3_LW fails at 2 waits,
TensorCopy/S4D4_TR at 2, Drain/CTRL_NO at 5 - TRN2 HW allows 1 per
bacc.generate_event_semaphores). Tile emits up to ~10 waits on the final
drain, so _cap_sync_waits() splits excess waits onto InstNoOp instructions
inserted just before the offender on the same engine.
"""

import sys

sys.path.insert(0, "/opt/trn_rl_repo")

import numpy as np

import concourse.bass as bass
import concourse.mybir as mybir
import concourse.tile as tile
from concourse.bass_utils import run_bass_kernel_spmd

F32 = mybir.dt.float32
FP16 = mybir.dt.float16

N_CORES = 8
IMGS_PER_CORE = 4
CIN = 128
COUT = 256
H = W = 56
HP = WP = 58  # padded plane
ROWS_PER_TILE = 8  # 8 output rows -> N = 448 <= 512 (one PSUM bank)
N_ROW_TILES = H // ROWS_PER_TILE  # 7
NTILE = ROWS_PER_TILE * W  # 448
N_WARMUP_MMS = 7
# dummy-MM free dim MUST match the real groups' 448 (same PSUM pool tag):
# a 512-wide dummy measured an overflow into a live accumulation bank
NWARM = NTILE

# Per-instruction sync-wait budget for the external walrus: TRN2 hardware
# allows at most 1 sync wait per instruction (bacc.generate_event_semaphores
# doc); observed failures: Matmult/S3_LW at 2, TensorCopy/S4D4_TR at 2,
# Drain/CTRL_NO at 5.
_WAIT_LIMITS_DEFAULT = 1
_WAIT_LIMITS = {}


def _cap_sync_waits(nc):
    """Split sync waits exceeding per-instruction limits onto same-engine
    InstNoOp instructions inserted immediately before the offender."""
    for fn in nc.m.functions:
        for bb in fn.blocks:
            i = 0
            insts = bb.instructions
            while i < len(insts):
                inst = insts[i]
                si = getattr(inst, "sync_info", None)
                if si is None or not si.on_wait:
                    i += 1
                    continue
                limit = _WAIT_LIMITS.get(type(inst).__name__, _WAIT_LIMITS_DEFAULT)
                waits = list(si.on_wait)
                if len(waits) <= limit:
                    i += 1
                    continue
                keep = waits[:limit]
                excess = waits[limit:]
                inst.sync_info = mybir.SyncInfo(
                    on_wait=keep, on_update=list(si.on_update)
                )
                pos = i
                for j in range(0, len(excess), _WAIT_LIMITS_DEFAULT):
                    chunk = excess[j : j + _WAIT_LIMITS_DEFAULT]
                    nop = mybir.InstNoOp(
                        name=nc.get_next_instruction_name(), ins=[], outs=[]
                    )
                    nop.engine = inst.engine
                    nop.sync_info = mybir.SyncInfo(on_wait=chunk, on_update=[])
                    nc.register_instruction(nop)
                    insts.insert(pos, nop)
                    pos += 1
                    i += 1
                i += 1


def build_conv_nc():
    """One-core program: x:(4,128,56,56) w/ wT16:(128,9,256) fp16,
    bias2:(128,2) -> out:(4,256,56,56)."""
    nc = bass.Bass()
    x = nc.dram_tensor("x", [IMGS_PER_CORE, CIN, H, W], F32, kind="ExternalInput")
    wt = nc.dram_tensor("wT16", [CIN, 9, COUT], FP16, kind="ExternalInput")
    bias2 = nc.dram_tensor("bias2", [128, 2], F32, kind="ExternalInput")
    out = nc.dram_tensor(
        "out", [IMGS_PER_CORE, COUT, H, W], F32, kind="ExternalOutput"
    )

    with tile.TileContext(nc) as tc:
        with (
            tc.tile_pool(name="const", bufs=1) as const_pool,
            tc.tile_pool(name="xpad", bufs=1) as xpad_pool,
            tc.tile_pool(name="xstage", bufs=6) as xstage_pool,
            tc.tile_pool(name="obuf", bufs=4) as obuf_pool,
            tc.tile_pool(name="psum", bufs=8, space="PSUM") as psum_pool,
        ):
            w_sb = const_pool.tile([CIN, 9 * COUT], FP16)
            b_sb = const_pool.tile([128, 2], F32)
            wz = const_pool.tile([128, NWARM], FP16)
            xpads = [
                xpad_pool.tile([CIN, HP, WP], FP16, tag=f"xpad{bi}", name=f"xpad{bi}")
                for bi in range(2)
            ]

            def zero_borders(xp):
                # Only the 1-wide borders need zeroing (interior is fully
                # overwritten per image). gpsimd memsets keep this off the
                # DVE queue.
                nc.gpsimd.memset(xp[:, 0, :], 0.0)            # top row
                nc.gpsimd.memset(xp[:, HP - 1, :], 0.0)       # bottom
                nc.gpsimd.memset(xp[:, 1 : HP - 1, 0], 0.0)   # left col
                nc.gpsimd.memset(xp[:, 1 : HP - 1, WP - 1], 0.0)  # right

            def x_tile(img, t):
                # x rides the sync HWDGE ring EXCLUSIVELY: the two HWDGE
                # rings drain with strict priority (sync starves scalar),
                # so x - the matmul-critical stream - must not share a ring
                # with the bulky weight/output transfers. DVE runs only
                # these casts, in consumption order.
                xp = xpads[img % 2]
                y0 = t * ROWS_PER_TILE
                xs = xstage_pool.tile(
                    [CIN, ROWS_PER_TILE, W], F32, tag="xs", name=f"xs_{img}_{t}"
                )
                nc.sync.dma_start(xs[:], x[img, :, y0 : y0 + ROWS_PER_TILE, :])
                nc.vector.tensor_copy(
                    xp[:, y0 + 1 : y0 + 1 + ROWS_PER_TILE, 1 : W + 1], xs[:]
                )

            # Startup, ordered for the PE ramp. The warmup tile memset runs
            # on gpsimd (whose body ops start earliest), so dummy matmuls
            # can begin right as the PE body starts and open the HAM
            # activity window ~2us before the real stream. Weights go on
            # the (lower-priority) scalar ring, split so early taps land
            # before the first group consumes them.
            nc.gpsimd.memset(wz[:], 0.0)
            # tile (0,0) is THE critical path to the first matmul: one
            # whole-tile DMA, first on the sync ring. (Splitting it, moving
            # it to SWDGE, or shipping it as a host-fp16 sidecar all
            # measured slower - extra/changed head DMAs couple semaphore
            # thresholds and delay the first cast.)
            x_tile(0, 0)
            # weight tap groups follow on the SAME ring - FIFO keeps x00
            # first - each landing just ahead of the matmuls that consume
            # it. bias rides the idle scalar ring.
            nc.sync.dma_start(w_sb[:, : 3 * COUT], wt[:, 0:3, :])       # taps 0-2
            # x01 rides between the weight groups: both t=0 PSUM groups'
            # dr=2 taps read padded rows 9-10 (= x01's first rows), so x01
            # behind all three weight DMAs measured a ~0.9us stream stall
            x_tile(0, 1)
            nc.sync.dma_start(w_sb[:, 3 * COUT : 6 * COUT], wt[:, 3:6, :])
            nc.sync.dma_start(w_sb[:, 6 * COUT :], wt[:, 6:, :])        # taps 6-8
            nc.scalar.dma_start(b_sb[:], bias2[:])
            zero_borders(xpads[0])
            dummy_ps = psum_pool.tile([128, NWARM], F32, tag="ps", name="ps_dummy")
            for k in range(N_WARMUP_MMS):
                nc.tensor.matmul(
                    dummy_ps[:],
                    wz[:, :128],
                    wz[:],
                    start=(k == 0),
                    stop=(k == N_WARMUP_MMS - 1),
                )
            x_tile(0, 2)
            x_tile(0, 3)
            zero_borders(xpads[1])
            for t in range(4, N_ROW_TILES):
                x_tile(0, t)

            def act_store(img, t, pss):
                # out = Identity(psum * 1.0 + bias[co]) on ScalarE, then
                # one batched DMA for both Cout chunks on the scalar ring
                # (keeping the sync ring free for x). The very last tile
                # is split into halves across both HWDGE rings to shorten
                # the tail.
                y0 = t * ROWS_PER_TILE
                last = img == IMGS_PER_CORE - 1 and t == N_ROW_TILES - 1
                if not last:
                    ob2 = obuf_pool.tile(
                        [128, 2, ROWS_PER_TILE, W], F32, tag="ob",
                        name=f"ob_{img}_{t}",
                    )
                    for c in range(2):
                        nc.scalar.activation(
                            ob2[:, c],
                            pss[c][:].rearrange("p (r w) -> p r w", w=W),
                            mybir.ActivationFunctionType.Identity,
                            bias=b_sb[:, c : c + 1],
                            scale=1.0,
                        )
                    ov = out[img].rearrange("(c p) h w -> p c h w", p=128)[
                        :, :, y0 : y0 + ROWS_PER_TILE, :
                    ]
                    nc.scalar.dma_start(ov, ob2[:])
                    return
                # final tile: chunk 0 (computed earlier) drains whole on
                # scalar; chunk 1 arrives as two 4-row PSUM groups whose
                # drains ride the (by now idle) sync ring, so the first
                # half's activation+store overlaps the second half's
                # matmuls.
                ob0 = obuf_pool.tile(
                    [128, ROWS_PER_TILE, W], F32, tag="obl", name=f"obl_{img}_{t}_0"
                )
                nc.scalar.activation(
                    ob0[:],
                    pss[0][:].rearrange("p (r w) -> p r w", w=W),
                    mybir.ActivationFunctionType.Identity,
                    bias=b_sb[:, 0:1],
                    scale=1.0,
                )
                nc.scalar.dma_start(
                    out[img, 0:128, y0 : y0 + ROWS_PER_TILE, :], ob0[:]
                )
                # 6-row then 2-row pieces: the very last piece's
                # activation+store chain is as short as possible, and the
                # two ride different HWDGE rings so their transfers +
                # completion receipts run in parallel
                for h, (py0, rows) in enumerate([(y0, 6), (y0 + 6, 2)]):
                    obh = obuf_pool.tile(
                        [128, rows, W], F32, tag="obl", name=f"obl_{img}_{t}_1_{h}"
                    )
                    nc.scalar.activation(
                        obh[:],
                        pss[1 + h][:].rearrange("p (r w) -> p r w", w=W),
                        mybir.ActivationFunctionType.Identity,
                        bias=b_sb[:, 1:2],
                        scale=1.0,
                    )
                    eng = nc.scalar if h == 0 else nc.sync
                    eng.dma_start(
                        out[img, 128:256, py0 : py0 + rows, :],
                        obh[:],
                    )

            for img in range(IMGS_PER_CORE):
                xp = xpads[img % 2]
                if img > 0:
                    for t in range(N_ROW_TILES):
                        x_tile(img, t)

                for t in range(N_ROW_TILES):
                    y0 = t * ROWS_PER_TILE
                    last = img == IMGS_PER_CORE - 1 and t == N_ROW_TILES - 1
                    # the final Cout chunk is computed as two 4-row PSUM
                    # groups so its drain pipeline is half as deep
                    groups = (
                        [(0, y0, ROWS_PER_TILE)]
                        + (
                            [(1, y0, 6), (1, y0 + 6, 2)]
                            if last
                            else [(1, y0, ROWS_PER_TILE)]
                        )
                    )
                    pss = []
                    for gi, (c, gy0, rows) in enumerate(groups):
                        ps = psum_pool.tile(
                            [128, rows * W], F32, tag="ps", name=f"ps_{img}_{t}_{gi}"
                        )
                        k = 0
                        for dr in range(3):
                            for dc in range(3):
                                lhsT = w_sb[
                                    :,
                                    (dr * 3 + dc) * COUT
                                    + c * 128 : (dr * 3 + dc) * COUT
                                    + c * 128
                                    + 128,
                                ]
                                rhs = xp[
                                    :,
                                    gy0 + dr : gy0 + dr + rows,
                                    dc : dc + W,
                                ]
                                nc.tensor.matmul(
                                    ps[:],
                                    lhsT,
                                    rhs,
                                    start=(k == 0),
                                    stop=(k == 8),
                                )
                                k += 1
                        pss.append(ps)
                    act_store(img, t, pss)

    _cap_sync_waits(nc)
    nc.finalize()
    return nc


_NC_CACHE = {}


def _get_nc():
    if "nc" not in _NC_CACHE:
        _NC_CACHE["nc"] = build_conv_nc()
    return _NC_CACHE["nc"]


def _prep_in_maps(x, weight, bias):
    x = np.ascontiguousarray(x, dtype=np.float32)
    # weight (256,128,3,3) -> wT16[ci, dr*3+dc, co], fp16 (the matmul dtype)
    wT16 = np.ascontiguousarray(
        np.transpose(np.asarray(weight, dtype=np.float32), (1, 2, 3, 0)).reshape(
            CIN, 9, COUT
        ).astype(np.float16)
    )
    bias2 = np.ascontiguousarray(
        np.asarray(bias, dtype=np.float32).reshape(2, 128).T
    )
    per_core = x.shape[0] // N_CORES
    return [
        {
            "x": x[i * per_core : (i + 1) * per_core],
            "wT16": wT16,
            "bias2": bias2,
        }
        for i in range(N_CORES)
    ]


def run(x, weight, bias, trace=False):
    """Run the conv on 8 cores; returns (out, BassKernelResults)."""
    nc = _get_nc()
    in_maps = _prep_in_maps(x, weight, bias)
    res = run_bass_kernel_spmd(
        nc, in_maps, core_ids=list(range(N_CORES)), trace=trace
    )
    out = np.concatenate([r["out"] for r in res.results], axis=0)
    return out, res


def kernel(x, weight, bias):
    out, _ = run(x, weight, bias, trace=False)
    return out


# revision 40
# speedup vs baseline: 1.1593x; 1.1593x over previous
"""Trainium2 Bass kernel for nn_Conv_6511170421767.

3x3 conv, stride 1, pad 1 on x:(32,128,56,56) with weight:(256,128,3,3),
bias:(256,) -> out:(32,256,56,56), fp32 in/out.

Strategy (data-parallel, 4 images per core on 8 cores):
- Cin=128 is exactly the PE contraction/partition dim. The conv becomes,
  per (output-row-block, Cout-chunk), an accumulation of 9 matmuls (one per
  kernel tap): out[co, pix] += W[dr,dc][ci,co].T @ xpad[ci, shifted pix].
- x is zero-padded once into SBUF as [128, 58, 58] per image; a matmul rhs
  slice [128, (8 rows x 58 stride), 56] walks the padded plane, so no edge
  fix-ups are needed. Borders are zeroed by gpsimd memsets (fp16); the
  interior is overwritten per image by DVE casts (f32 stage -> fp16 pad).
- Matmul operands are fp16 (1 PE cycle/row vs 4 for fp32; operand ranges
  sit safely inside fp16's dynamic range; measured rel err 2.9e-4 vs the
  fp32 reference). Weights are converted to fp16 on the host (a 1.1MB
  constant) and DMA'd directly - no on-device weight casts.
- PSUM tile [128, 448] = one bank; 9 taps accumulate in-bank, then the
  scalar engine adds bias (Identity activation w/ per-partition bias AP)
  while copying PSUM->SBUF, and the result DMAs out on the sync queue.

Schedule notes (from iterative trace analysis; baseline was 117.7us,
this version ~112.9us):
- The MM stream runs at the fp16 streaming wall (~191ns per N=448
  matmul) once warm; all the headroom is at the edges:
  - The two HWDGE rings drain with strict priority (sync starves
    scalar), and a DMA trigger occupies its sequencer ~0.6-0.8us. So the
    matmul-critical x stream gets the sync ring, with x00 first, then
    the weight tap groups (0-2/3-5/6-8, each landing just ahead of its
    consumers), then the remaining x tiles. Outputs (batched per
    row-tile via a rearranged DRAM AP) + activations + bias ride the
    scalar ring. DVE runs only the 28 x casts, in consumption order.
  - The PE HAM clock-gate needs ~3.4us of sustained busy to lift the
    engine from 1.2 to 2.4GHz, and a ragged early stream postpones the
    flip (~4.8us lost in the baseline). 7 dummy matmuls on a zeroed fp16
    tile start as soon as the PE body starts (~7.4us) and bridge to the
    first real MM (~10.8us), so the flip lands right as the real stream
    begins. (PE idle >~1.5us between dummies and stream resets the HAM
    window - the bridge must be gap-free - but each surplus dummy also
    delays the stream ~0.37us, so 7 is the measured sweet spot.)
  - Tail: the final Cout chunk is computed as two 4-row PSUM groups so
    its activation+store pipeline is half as deep, with the two output
    DMAs split across both HWDGE rings.
  - Head DMAs must stay few and decoupled: splitting x00 into halves or
    moving it to SWDGE measured SLOWER (semaphore-threshold coupling
    delays the first cast; SWDGE adds ~1us emission+receipt).

The external neuronxcc walrus in this container enforces small per-
instruction sync-wait limits (Matmult/S3_LW fails at 2 waits,
TensorCopy/S4D4_TR at 2, Drain/CTRL_NO at 5 - TRN2 HW allows 1 per
bacc.generate_event_semaphores). Tile emits up to ~10 waits on the final
drain, so _cap_sync_waits() splits excess waits onto InstNoOp instructions
inserted just before the offender on the same engine.
"""

import sys

sys.path.insert(0, "/opt/trn_rl_repo")

import numpy as np

import concourse.bass as bass
import concourse.mybir as mybir
import concourse.tile as tile
from concourse.bass_utils import run_bass_kernel_spmd

F32 = mybir.dt.float32
FP16 = mybir.dt.float16

N_CORES = 8
IMGS_PER_CORE = 4
CIN = 128
COUT = 256
H = W = 56
HP = WP = 58  # padded plane
ROWS_PER_TILE = 8  # 8 output rows -> N = 448 <= 512 (one PSUM bank)
N_ROW_TILES = H // ROWS_PER_TILE  # 7
NTILE = ROWS_PER_TILE * W  # 448
N_WARMUP_MMS = 7
# dummy-MM free dim MUST match the real groups' 448 (same PSUM pool tag):
# a 512-wide dummy measured an overflow into a live accumulation bank
NWARM = NTILE

# Per-instruction sync-wait budget for the external walrus: TRN2 hardware
# allows at most 1 sync wait per instruction (bacc.generate_event_semaphores
# doc); observed failures: Matmult/S3_LW at 2, TensorCopy/S4D4_TR at 2,
# Drain/CTRL_NO at 5.
_WAIT_LIMITS_DEFAULT = 1
_WAIT_LIMITS = {}


def _cap_sync_waits(nc):
    """Split sync waits exceeding per-instruction limits onto same-engine
    InstNoOp instructions inserted immediately before the offender."""
    for fn in nc.m.functions:
        for bb in fn.blocks:
            i = 0
            insts = bb.instructions
            while i < len(insts):
                inst = insts[i]
                si = getattr(inst, "sync_info", None)
                if si is None or not si.on_wait:
                    i += 1
                    continue
                limit = _WAIT_LIMITS.get(type(inst).__name__, _WAIT_LIMITS_DEFAULT)
                waits = list(si.on_wait)
                if len(waits) <= limit:
                    i += 1
                    continue
                keep = waits[:limit]
                excess = waits[limit:]
                inst.sync_info = mybir.SyncInfo(
                    on_wait=keep, on_update=list(si.on_update)
                )
                pos = i
                for j in range(0, len(excess), _WAIT_LIMITS_DEFAULT):
                    chunk = excess[j : j + _WAIT_LIMITS_DEFAULT]
                    nop = mybir.InstNoOp(
                        name=nc.get_next_instruction_name(), ins=[], outs=[]
                    )
                    nop.engine = inst.engine
                    nop.sync_info = mybir.SyncInfo(on_wait=chunk, on_update=[])
                    nc.register_instruction(nop)
                    insts.insert(pos, nop)
                    pos += 1
                    i += 1
                i += 1


def _strip_const_ap_memsets(nc):
    """Remove the framework's const-AP init memsets (const-float32-0.0 etc)
    from the preamble block: nothing in this kernel reads them, they carry
    no sync_info, and they run on Pool BEFORE the all-engine gather barrier
    that Pool itself coordinates - so they delay every engine's body start
    by ~0.4us."""
    bb = nc.m.functions[0].blocks[0]
    keep = []
    for inst in bb.instructions:
        if type(inst).__name__ == "InstMemset":
            memrefs = [
                str(getattr(o, "memref", "")) for o in getattr(inst, "outs", [])
            ]
            if any(m.startswith("const-") for m in memrefs):
                continue
        keep.append(inst)
    bb.instructions[:] = keep


def build_conv_nc():
    """One-core program: x:(4,128,56,56) w/ wT16:(128,9,256) fp16,
    bias2:(128,2) -> out:(4,256,56,56)."""
    nc = bass.Bass()
    x = nc.dram_tensor("x", [IMGS_PER_CORE, CIN, H, W], F32, kind="ExternalInput")
    wt = nc.dram_tensor("wT16", [CIN, 9, COUT], FP16, kind="ExternalInput")
    bias2 = nc.dram_tensor("bias2", [128, 2], F32, kind="ExternalInput")
    out = nc.dram_tensor(
        "out", [IMGS_PER_CORE, COUT, H, W], F32, kind="ExternalOutput"
    )

    with tile.TileContext(nc) as tc:
        with (
            tc.tile_pool(name="const", bufs=1) as const_pool,
            tc.tile_pool(name="xpad", bufs=1) as xpad_pool,
            tc.tile_pool(name="xstage", bufs=6) as xstage_pool,
            tc.tile_pool(name="obuf", bufs=4) as obuf_pool,
            tc.tile_pool(name="psum", bufs=8, space="PSUM") as psum_pool,
        ):
            w_sb = const_pool.tile([CIN, 9 * COUT], FP16)
            b_sb = const_pool.tile([128, 2], F32)
            wz = const_pool.tile([128, NWARM], FP16)
            xpads = [
                xpad_pool.tile([CIN, HP, WP], FP16, tag=f"xpad{bi}", name=f"xpad{bi}")
                for bi in range(2)
            ]

            def zero_borders(xp):
                # Only the 1-wide borders need zeroing (interior is fully
                # overwritten per image). gpsimd memsets keep this off the
                # DVE queue.
                nc.gpsimd.memset(xp[:, 0, :], 0.0)            # top row
                nc.gpsimd.memset(xp[:, HP - 1, :], 0.0)       # bottom
                nc.gpsimd.memset(xp[:, 1 : HP - 1, 0], 0.0)   # left col
                nc.gpsimd.memset(xp[:, 1 : HP - 1, WP - 1], 0.0)  # right

            def x_tile(img, t):
                # x rides the sync HWDGE ring EXCLUSIVELY: the two HWDGE
                # rings drain with strict priority (sync starves scalar),
                # so x - the matmul-critical stream - must not share a ring
                # with the bulky weight/output transfers. DVE runs only
                # these casts, in consumption order.
                xp = xpads[img % 2]
                y0 = t * ROWS_PER_TILE
                xs = xstage_pool.tile(
                    [CIN, ROWS_PER_TILE, W], F32, tag="xs", name=f"xs_{img}_{t}"
                )
                nc.sync.dma_start(xs[:], x[img, :, y0 : y0 + ROWS_PER_TILE, :])
                nc.vector.tensor_copy(
                    xp[:, y0 + 1 : y0 + 1 + ROWS_PER_TILE, 1 : W + 1], xs[:]
                )

            # Startup, ordered for the PE ramp. The warmup tile memset runs
            # on gpsimd (whose body ops start earliest), so dummy matmuls
            # can begin right as the PE body starts and open the HAM
            # activity window ~2us before the real stream. Weights go on
            # the (lower-priority) scalar ring, split so early taps land
            # before the first group consumes them.
            nc.gpsimd.memset(wz[:], 0.0)
            # tile (0,0) is THE critical path to the first matmul: one
            # whole-tile DMA, first on the sync ring. (Splitting it, moving
            # it to SWDGE, or shipping it as a host-fp16 sidecar all
            # measured slower - extra/changed head DMAs couple semaphore
            # thresholds and delay the first cast.)
            x_tile(0, 0)
            # weight tap groups follow on the SAME ring - FIFO keeps x00
            # first - each landing just ahead of the matmuls that consume
            # it. bias rides the idle scalar ring.
            nc.sync.dma_start(w_sb[:, : 3 * COUT], wt[:, 0:3, :])       # taps 0-2
            # x01 rides between the weight groups: both t=0 PSUM groups'
            # dr=2 taps read padded rows 9-10 (= x01's first rows), so x01
            # behind all three weight DMAs measured a ~0.9us stream stall
            x_tile(0, 1)
            nc.sync.dma_start(w_sb[:, 3 * COUT : 6 * COUT], wt[:, 3:6, :])
            nc.sync.dma_start(w_sb[:, 6 * COUT :], wt[:, 6:, :])        # taps 6-8
            nc.scalar.dma_start(b_sb[:], bias2[:])
            zero_borders(xpads[0])
            dummy_ps = psum_pool.tile([128, NWARM], F32, tag="ps", name="ps_dummy")
            for k in range(N_WARMUP_MMS):
                nc.tensor.matmul(
                    dummy_ps[:],
                    wz[:, :128],
                    wz[:],
                    start=(k == 0),
                    stop=(k == N_WARMUP_MMS - 1),
                )
            x_tile(0, 2)
            x_tile(0, 3)
            zero_borders(xpads[1])
            for t in range(4, N_ROW_TILES):
                x_tile(0, t)

            def act_store(img, t, pss):
                # out = Identity(psum * 1.0 + bias[co]) on ScalarE, then
                # one batched DMA for both Cout chunks on the scalar ring
                # (keeping the sync ring free for x). The very last tile
                # is split into halves across both HWDGE rings to shorten
                # the tail.
                y0 = t * ROWS_PER_TILE
                last = img == IMGS_PER_CORE - 1 and t == N_ROW_TILES - 1
                if not last:
                    ob2 = obuf_pool.tile(
                        [128, 2, ROWS_PER_TILE, W], F32, tag="ob",
                        name=f"ob_{img}_{t}",
                    )
                    for c in range(2):
                        nc.scalar.activation(
                            ob2[:, c],
                            pss[c][:].rearrange("p (r w) -> p r w", w=W),
                            mybir.ActivationFunctionType.Identity,
                            bias=b_sb[:, c : c + 1],
                            scale=1.0,
                        )
                    ov = out[img].rearrange("(c p) h w -> p c h w", p=128)[
                        :, :, y0 : y0 + ROWS_PER_TILE, :
                    ]
                    nc.scalar.dma_start(ov, ob2[:])
                    return
                # final tile: chunk 0 (computed earlier) drains whole on
                # scalar; chunk 1 arrives as two 4-row PSUM groups whose
                # drains ride the (by now idle) sync ring, so the first
                # half's activation+store overlaps the second half's
                # matmuls.
                ob0 = obuf_pool.tile(
                    [128, ROWS_PER_TILE, W], F32, tag="obl", name=f"obl_{img}_{t}_0"
                )
                nc.scalar.activation(
                    ob0[:],
                    pss[0][:].rearrange("p (r w) -> p r w", w=W),
                    mybir.ActivationFunctionType.Identity,
                    bias=b_sb[:, 0:1],
                    scale=1.0,
                )
                nc.scalar.dma_start(
                    out[img, 0:128, y0 : y0 + ROWS_PER_TILE, :], ob0[:]
                )
                # 6-row then 2-row pieces: the very last piece's
                # activation+store chain is as short as possible, and the
                # two ride different HWDGE rings so their transfers +
                # completion receipts run in parallel
                for h, (py0, rows) in enumerate([(y0, 6), (y0 + 6, 2)]):
                    obh = obuf_pool.tile(
                        [128, rows, W], F32, tag="obl", name=f"obl_{img}_{t}_1_{h}"
                    )
                    nc.scalar.activation(
                        obh[:],
                        pss[1 + h][:].rearrange("p (r w) -> p r w", w=W),
                        mybir.ActivationFunctionType.Identity,
                        bias=b_sb[:, 1:2],
                        scale=1.0,
                    )
                    eng = nc.scalar if h == 0 else nc.sync
                    eng.dma_start(
                        out[img, 128:256, py0 : py0 + rows, :],
                        obh[:],
                    )

            for img in range(IMGS_PER_CORE):
                xp = xpads[img % 2]
                if img > 0:
                    for t in range(N_ROW_TILES):
                        x_tile(img, t)

                for t in range(N_ROW_TILES):
                    y0 = t * ROWS_PER_TILE
                    last = img == IMGS_PER_CORE - 1 and t == N_ROW_TILES - 1
                    # the final Cout chunk is computed as two 4-row PSUM
                    # groups so its drain pipeline is half as deep
                    groups = (
                        [(0, y0, ROWS_PER_TILE)]
                        + (
                            [(1, y0, 6), (1, y0 + 6, 2)]
                            if last
                            else [(1, y0, ROWS_PER_TILE)]
                        )
                    )
                    pss = []
                    for gi, (c, gy0, rows) in enumerate(groups):
                        ps = psum_pool.tile(
                            [128, rows * W], F32, tag="ps", name=f"ps_{img}_{t}_{gi}"
                        )
                        k = 0
                        for dr in range(3):
                            for dc in range(3):
                                lhsT = w_sb[
                                    :,
                                    (dr * 3 + dc) * COUT
                                    + c * 128 : (dr * 3 + dc) * COUT
                                    + c * 128
                                    + 128,
                                ]
                                rhs = xp[
                                    :,
                                    gy0 + dr : gy0 + dr + rows,
                                    dc : dc + W,
                                ]
                                nc.tensor.matmul(
                                    ps[:],
                                    lhsT,
                                    rhs,
                                    start=(k == 0),
                                    stop=(k == 8),
                                )
                                k += 1
                        pss.append(ps)
                    act_store(img, t, pss)

    _cap_sync_waits(nc)
    _strip_const_ap_memsets(nc)
    nc.finalize()
    return nc


_NC_CACHE = {}


def _get_nc():
    if "nc" not in _NC_CACHE:
        _NC_CACHE["nc"] = build_conv_nc()
    return _NC_CACHE["nc"]


def _prep_in_maps(x, weight, bias):
    x = np.ascontiguousarray(x, dtype=np.float32)
    # weight (256,128,3,3) -> wT16[ci, dr*3+dc, co], fp16 (the matmul dtype)
    wT16 = np.ascontiguousarray(
        np.transpose(np.asarray(weight, dtype=np.float32), (1, 2, 3, 0)).reshape(
            CIN, 9, COUT
        ).astype(np.float16)
    )
    bias2 = np.ascontiguousarray(
        np.asarray(bias, dtype=np.float32).reshape(2, 128).T
    )
    per_core = x.shape[0] // N_CORES
    return [
        {
            "x": x[i * per_core : (i + 1) * per_core],
            "wT16": wT16,
            "bias2": bias2,
        }
        for i in range(N_CORES)
    ]


def run(x, weight, bias, trace=False):
    """Run the conv on 8 cores; returns (out, BassKernelResults)."""
    nc = _get_nc()
    in_maps = _prep_in_maps(x, weight, bias)
    res = run_bass_kernel_spmd(
        nc, in_maps, core_ids=list(range(N_CORES)), trace=trace
    )
    out = np.concatenate([r["out"] for r in res.results], axis=0)
    return out, res


def kernel(x, weight, bias):
    out, _ = run(x, weight, bias, trace=False)
    return out
